# revision 10
# baseline (speedup 1.0000x reference)
"""Trainium2 Bass kernel for windowed mean-pooling (segment_reduce).

Computes, for each (batch b, window w):
    out[b, w, :] = mean over t in [begins[b,w], ends'[b,w]) of features[b, t, :]
where ends' = clip(ends, begins, begins + 8) (the reference gathers at most
MAX_WINDOW=8 tokens) and empty windows produce 0 (count clamped to >= 1).

Strategy (data-parallel over batch, one sample per NeuronCore). The kernel is
HBM-bound; v2 minimizes bytes AND engine work on the critical path:
  - features ship as fp16 [P, NKT, D] (token t on partition t%128, K-tile
    t//128; 6.3 MB), streamed over the two HWDGE rings (ACT+SP) in chunks
    with per-partition-contiguous lines up to 12 KB.
  - window masks S[t, w] = (begins[w] <= t < ends[w]) are built ON THE HOST
    and shipped as 0/1 fp8 (1 MB for the union strip layout) -- this deletes
    the whole on-device metadata pipeline (broadcast matmuls, casts, 19 us
    of vector compares) that limited the baseline. The PE consumes the fp8
    mask directly as the stationary operand against fp16 features.
  - out_block = S^T @ F accumulated over K-tiles in PSUM (512+256 col split),
    scaled by 1/count via per-partition activation scale on ACT, written as
    fp16 into a [P, NBLK, D] DRAM buffer (host un-shuffles + upcasts);
    output DMAs batch 4 blocks (6 KB lines) early, smaller at the tail.
  - a dozen dummy warm-up matmuls at t=0 ramp the PE out of its low p-state
    (0.65 -> 2.4 GHz takes ~3 us of continuous execution) while DMAs stream.
"""

import os
import sys

import numpy as np

for _p in ("/opt/trn_rl_repo", "/root/.axon_site/_ro/trn_rl_repo"):
    if os.path.isdir(_p) and _p not in sys.path:
        sys.path.insert(0, _p)

import ml_dtypes  # noqa: E402

from concourse import bacc, mybir  # noqa: E402
import concourse.tile as tile  # noqa: E402
from concourse.bass_utils import run_bass_kernel_spmd  # noqa: E402

B, T, D, W = 8, 4096, 768, 2048
MAXWIN = 8
P = 128
NBLK = W // P  # 16 window blocks of 128 windows
NKT = T // P  # 32 K-tiles of 128 tokens
FCHUNKS = (1, 1, 2, 4, 4, 4, 8, 8)  # K-tiles per feature DMA chunk
OGROUPS = (4, 4, 4, 2, 1, 1)  # output blocks per output DMA
NWARM = 5  # PE warm-up matmuls ([P, 512] each)
KSPLIT = 6  # mask strips for k < KSPLIT ship in the head DMA
F32 = mybir.dt.float32
FP16 = mybir.dt.float16
FP8 = mybir.dt.float8e4

FP8NP = ml_dtypes.float8_e4m3


def _strip_layout(klo, khi):
    """Column layout of the union mask strips: for each K-tile k, the blocks
    [blo, bhi) that consume it, at column offset off (128 cols per block)."""
    strips = {}
    off = 0
    for k in range(NKT):
        blks = [i for i in range(NBLK) if klo[i] <= k < khi[i]]
        if blks:
            blo, bhi = min(blks), max(blks) + 1
            strips[k] = (blo, bhi, off)
            off += (bhi - blo) * P
    return strips, off


def _build_program(klo, khi):
    """Build the SPMD Bass program given per-block K-tile ranges [klo, khi)."""
    strips, mw = _strip_layout(klo, khi)
    nc = bacc.Bacc(None)

    fhi_d = nc.declare_dram_parameter("fhi", [P, NKT, D], FP16, isOutput=False)
    mask_d = nc.declare_dram_parameter("mask8", [P, mw], FP8, isOutput=False)
    iv_d = nc.declare_dram_parameter("iv", [P, P], F32, isOutput=False)
    out_d = nc.declare_dram_parameter("out", [P, NBLK, D], FP16, isOutput=True)

    fhi_r = fhi_d[:]
    out_r = out_d[:]

    with tile.TileContext(nc) as tc:
        with (
            tc.tile_pool(name="warmp", bufs=1) as warm_pool,
            tc.tile_pool(name="metap", bufs=1) as meta_pool,
            tc.tile_pool(name="fslab", bufs=1) as f_pool,
            tc.tile_pool(name="outp", bufs=2) as out_pool,
            tc.tile_pool(name="psum", bufs=4, space="PSUM") as psum_pool,
        ):
            # --- PE warm-up: ramp the tensor engine p-state while DMAs run.
            warm_sb = warm_pool.tile([P, 512], FP16)
            nc.vector.memset(warm_sb[:], 0.0)
            for j in range(NWARM):
                wp = psum_pool.tile([P, 512], F32, name=f"warm{j}", tag="ps")
                nc.tensor.matmul(
                    wp[:], warm_sb[:, 0:P], warm_sb[:], start=True, stop=True
                )

            # --- metadata: 1/count per (block, partition) + the mask strips.
            # The mask head (strips for k < KSPLIT, i.e. the first blocks)
            # ships before most feature chunks so the PE starts early; no
            # compute runs on the Activation engine, so its HWDGE ring
            # dispatches from t=0 (no ACT_TABLE_LOAD in front).
            head_end = mw
            for k in sorted(strips):
                if k >= KSPLIT:
                    head_end = strips[k][2]
                    break
            # masks + iv ride the GPSIMD SWDGE ring so the two HWDGE rings
            # carry nothing but features (in K order) and outputs. The mask
            # ships in per-K-group DMAs aligned to the feature chunks so
            # LDWEIGHTS for K-tile k waits only on its own small DMA, not
            # the whole 1 MB mask transfer.
            mask_sb = meta_pool.tile([P, mw], FP8)
            iv_sb = meta_pool.tile([P, P], F32)
            mgroups = []  # column split points aligned to FCHUNK boundaries
            k0 = 0
            for sz in FCHUNKS:
                k0 += sz
                nxt = mw
                for k in sorted(strips):
                    if k >= k0:
                        nxt = strips[k][2]
                        break
                if not mgroups or nxt > mgroups[-1]:
                    mgroups.append(nxt)
            c0 = 0
            for gj, c1 in enumerate(mgroups):
                nc.gpsimd.dma_start(out=mask_sb[:, c0:c1], in_=mask_d[:][:, c0:c1])
                if gj == 0:
                    nc.gpsimd.dma_start(out=iv_sb[:], in_=iv_d[:])
                c0 = c1
            assert c0 == mw

            # --- feature slab chunks (fp16), alternating HWDGE rings.
            fhi_tiles = []
            k2chunk = []
            k0 = 0
            for j, sz in enumerate(FCHUNKS):
                fh = f_pool.tile([P, sz, D], FP16, name=f"fh{j}", tag=f"fh{j}")
                eng = nc.scalar if j % 2 == 0 else nc.sync
                eng.dma_start(out=fh[:], in_=fhi_r[:, k0 : k0 + sz, :])
                fhi_tiles.append(fh)
                for s in range(sz):
                    k2chunk.append((j, s))
                k0 += sz
            assert k0 == NKT

            # --- block matmuls + evacuation + grouped output DMA.
            og_starts = []
            o0 = 0
            for g in OGROUPS:
                og_starts.append(o0)
                o0 += g
            assert o0 == NBLK

            gi = 0
            os_tile = None
            for i in range(NBLK):
                if i == og_starts[gi]:
                    os_tile = out_pool.tile(
                        [P, OGROUPS[gi], D], FP16, name=f"os{gi}", tag="os"
                    )
                ps = psum_pool.tile([P, D], F32, name=f"ps{i}", tag="ps")
                for k in range(klo[i], khi[i]):
                    blo, bhi, off = strips[k]
                    lh = mask_sb[:, off + (i - blo) * P : off + (i - blo + 1) * P]
                    cj, cs = k2chunk[k]
                    rh = fhi_tiles[cj][:, cs, :]
                    first = k == klo[i]
                    last = k == khi[i] - 1
                    for n0, nn in ((0, 512), (512, 256)):
                        nc.tensor.matmul(
                            ps[:, n0 : n0 + nn], lh, rh[:, n0 : n0 + nn],
                            start=first, stop=(last and n0 == 512),
                        )
                # evacuate PSUM alternating ACT/DVE so evacuation throughput
                # (~1 us per block on one engine) never paces the PE tail.
                if i % 2 == 0:
                    nc.scalar.mul(
                        out=os_tile[:, i - og_starts[gi], :], in_=ps[:],
                        mul=iv_sb[:, i : i + 1],
                    )
                else:
                    nc.vector.tensor_scalar(
                        os_tile[:, i - og_starts[gi], :], ps[:],
                        iv_sb[:, i : i + 1], None, mybir.AluOpType.mult,
                    )
                if i == og_starts[gi] + OGROUPS[gi] - 1:
                    eng = nc.scalar if gi % 2 == 0 else nc.sync
                    eng.dma_start(
                        out=out_r[:, og_starts[gi] : i + 1, :], in_=os_tile[:]
                    )
                    gi += 1

    nc.finalize()
    return nc


def _prepare(features, begins, ends):
    feats = np.asarray(features, dtype=np.float32)
    assert feats.shape == (B, T, D), feats.shape
    b = np.clip(np.asarray(begins).astype(np.int64), 0, T - 1)
    e = np.asarray(ends).astype(np.int64)
    # Reference gathers at most MAXWIN tokens starting at b; empty -> count 1.
    e_eff = np.clip(e, b, np.minimum(b + MAXWIN, T))
    counts = np.maximum(e_eff - b, 1).astype(np.float32)
    inv = (1.0 / counts).astype(np.float32)

    bw = b.reshape(B, NBLK, P)
    ew = e_eff.reshape(B, NBLK, P)
    klo_pc = bw.min(-1) // P  # [B, NBLK]
    khi_pc = (np.maximum(ew.max(-1) - 1, bw.min(-1)) // P) + 1
    klo = klo_pc.min(0).astype(int)
    khi = khi_pc.max(0).astype(int)
    khi = np.minimum(np.maximum(khi, klo + 1), NKT)
    klo, khi = list(klo), list(khi)

    strips, mw = _strip_layout(klo, khi)

    # shuffle to [P, NKT, D]: partition p holds tokens {p, 128+p, ...}
    hi = np.ascontiguousarray(
        feats.astype(np.float16).reshape(B, NKT, P, D).transpose(0, 2, 1, 3)
    )

    # 0/1 masks in the union strip layout (fp8: 0/1 exact).
    tok = np.arange(NKT * P).reshape(NKT, P)  # tok[k, p] = 128k + p
    mask8 = np.zeros((B, P, mw), dtype=FP8NP)
    for k, (blo, bhi, off) in strips.items():
        wlo, whi = blo * P, bhi * P
        t_col = tok[k][:, None]  # [P, 1]
        m = (b[:, wlo:whi][:, None, :] <= t_col) & (
            t_col < e_eff[:, wlo:whi][:, None, :]
        )  # [B, P, wn]
        mask8[:, :, off : off + (whi - wlo)] = m.astype(FP8NP)

    in_maps = []
    for c in range(B):
        iv = np.zeros((P, P), np.float32)
        iv[:, 0:NBLK] = inv[c].reshape(NBLK, P).T
        in_maps.append({"fhi": hi[c], "mask8": mask8[c], "iv": iv})
    return klo, khi, in_maps


def run(features, begins, ends, trace=False):
    """Build + run on 8 NeuronCores; returns (output, BassKernelResults)."""
    klo, khi, in_maps = _prepare(features, begins, ends)
    nc = _build_program(klo, khi)
    res = run_bass_kernel_spmd(nc, in_maps, list(range(B)), trace=trace)
    # out is [P, NBLK, D] fp16 with window w = i*128 + p at [p, i, :]
    out = np.stack(
        [
            np.ascontiguousarray(
                res.results[c]["out"].astype(np.float32).transpose(1, 0, 2)
            ).reshape(W, D)
            for c in range(B)
        ],
        axis=0,
    )
    return out, res


def kernel(features, begins, ends):
    out, _ = run(features, begins, ends, trace=False)
    return out


# revision 13
# speedup vs baseline: 1.1504x; 1.1504x over previous
"""Trainium2 Bass kernel for windowed mean-pooling (segment_reduce).

Computes, for each (batch b, window w):
    out[b, w, :] = mean over t in [begins[b,w], ends'[b,w]) of features[b, t, :]
where ends' = clip(ends, begins, begins + 8) (the reference gathers at most
MAX_WINDOW=8 tokens) and empty windows produce 0 (count clamped to >= 1).

Strategy (data-parallel over batch, one sample per NeuronCore). The kernel is
HBM/DMA-queue bound: each DMA descriptor line costs ~87 ns + bytes/43 GB/s on
one of 16 queues, so the design minimizes total line count x overhead:
  - features fp16 [P, NKT, D] (token t on partition t%128) in chunks
    (2,2,4,8,8,8) K-tiles -> per-partition lines of 3-12 KB, split across
    the two HWDGE rings (ACT + SP) in K order.
  - masks are built ON DEVICE from a 8 KB metadata row (begins/ends fp16,
    broadcast across partitions by K=1 ones-matmuls, compare ops split
    between DVE and GpSimd) -- 1 MB of host masks would cost ~7 us of queue
    time. Exception: strips for k < KSPLIT ship as a small host-built fp8
    mask head so the first blocks' matmuls skip the broadcast/compare
    latency chain (the PE consumes fp8 stationary vs fp16 moving directly).
  - out_block = S^T @ F accumulated in PSUM (512+256 col split), scaled by
    1/count on ACT (per-partition activation scale), written fp16 to a
    [P, NBLK, D] DRAM buffer in groups (8,4,2,1,1) -> 12 KB lines early,
    1.5 KB at the latency-critical tail; host un-shuffles + upcasts.
  - warm-up matmuls at t=0 ramp the PE p-state (0.65 -> 2.4 GHz needs ~3 us
    of continuous execution) before the real matmul stream begins.
"""

import os
import sys

import numpy as np

for _p in ("/opt/trn_rl_repo", "/root/.axon_site/_ro/trn_rl_repo"):
    if os.path.isdir(_p) and _p not in sys.path:
        sys.path.insert(0, _p)

import ml_dtypes  # noqa: E402

from concourse import bacc, mybir  # noqa: E402
import concourse.tile as tile  # noqa: E402
from concourse.bass_utils import run_bass_kernel_spmd  # noqa: E402

B, T, D, W = 8, 4096, 768, 2048
MAXWIN = 8
P = 128
NBLK = W // P  # 16 window blocks of 128 windows
NKT = T // P  # 32 K-tiles of 128 tokens
FCHUNKS = (2, 2, 4, 8, 8, 8)  # K-tiles per feature DMA chunk
OGROUPS = (8, 4, 2, 1, 1)  # output blocks per output DMA
NWARM = 5  # PE warm-up matmuls ([P, 512] each)
KSPLIT = 12  # strips for k < KSPLIT ship as host fp8 masks
MCH = 512  # windows per metadata broadcast matmul
F32 = mybir.dt.float32
FP16 = mybir.dt.float16
FP8 = mybir.dt.float8e4

FP8NP = ml_dtypes.float8_e4m3


def _strip_layout(klo, khi):
    """For each K-tile k, the block span [blo, bhi) that consumes it; strips
    with k < KSPLIT also get a column offset in the host fp8 mask head."""
    strips = {}
    off = 0
    for k in range(NKT):
        blks = [i for i in range(NBLK) if klo[i] <= k < khi[i]]
        if blks:
            blo, bhi = min(blks), max(blks) + 1
            hoff = off if k < KSPLIT else None
            strips[k] = (blo, bhi, hoff)
            if k < KSPLIT:
                off += (bhi - blo) * P
    return strips, off


def _build_program(klo, khi):
    """Build the SPMD Bass program given per-block K-tile ranges [klo, khi)."""
    strips, hw8 = _strip_layout(klo, khi)
    nc = bacc.Bacc(None)

    fhi_d = nc.declare_dram_parameter("fhi", [P, NKT, D], FP16, isOutput=False)
    m8_d = nc.declare_dram_parameter("mask8h", [P, hw8], FP8, isOutput=False)
    meta_d = nc.declare_dram_parameter("meta", [1, 2, W], FP16, isOutput=False)
    ioiv_d = nc.declare_dram_parameter("ioiv", [P, 64], F32, isOutput=False)
    out_d = nc.declare_dram_parameter("out", [P, NBLK, D], FP16, isOutput=True)

    fhi_r = fhi_d[:]
    out_r = out_d[:]

    with tile.TileContext(nc) as tc:
        with (
            tc.tile_pool(name="warmp", bufs=1) as warm_pool,
            tc.tile_pool(name="metap", bufs=1) as meta_pool,
            tc.tile_pool(name="fslab", bufs=1) as f_pool,
            tc.tile_pool(name="m2p", bufs=4) as m2_pool,
            tc.tile_pool(name="maskp", bufs=12) as mask_pool,
            tc.tile_pool(name="outp", bufs=2) as out_pool,
            tc.tile_pool(name="psum", bufs=4, space="PSUM") as psum_pool,
        ):
            # --- PE warm-up: ramp the tensor engine p-state while DMAs start.
            warm_sb = warm_pool.tile([P, 512], FP16)
            nc.vector.memset(warm_sb[:], 0.0)
            for j in range(NWARM):
                wp = psum_pool.tile([P, 512], F32, name=f"warm{j}", tag="ps")
                nc.tensor.matmul(
                    wp[:], warm_sb[:, 0:P], warm_sb[:], start=True, stop=True
                )

            # --- metadata DMAs (tiny, first on each ring).
            m8_sb = meta_pool.tile([P, hw8], FP8)
            nc.scalar.dma_start(out=m8_sb[:], in_=m8_d[:])
            meta_sb = meta_pool.tile([1, 2, W], FP16)
            nc.sync.dma_start(out=meta_sb[:], in_=meta_d[:])
            ioiv_sb = meta_pool.tile([P, 64], F32)
            nc.sync.dma_start(out=ioiv_sb[:], in_=ioiv_d[:])
            io_sb = ioiv_sb[:, 0:NKT]
            iv_sb = ioiv_sb[:, NKT : NKT + NBLK]

            # --- feature slab chunks (fp16), alternating HWDGE rings.
            fhi_tiles = []
            k2chunk = []
            k0 = 0
            for j, sz in enumerate(FCHUNKS):
                fh = f_pool.tile([P, sz, D], FP16, name=f"fh{j}", tag=f"fh{j}")
                eng = nc.scalar if j % 2 == 0 else nc.sync
                eng.dma_start(out=fh[:], in_=fhi_r[:, k0 : k0 + sz, :])
                fhi_tiles.append(fh)
                for s in range(sz):
                    k2chunk.append((j, s))
                k0 += sz
            assert k0 == NKT

            # --- broadcast begins/ends across partitions via K=1 matmuls
            # with a ones row, evacuating each PSUM chunk as an fp16 cast on
            # DVE (values are -2048-shifted so fp16 is exact).
            # Only window chunks consumed by device-built strips need the
            # broadcast.
            dks = [k for k in sorted(strips) if strips[k][2] is None]
            smin = min(strips[k][0] * P for k in dks) // MCH if dks else 0
            smax = (
                (max(strips[k][1] * P for k in dks) + MCH - 1) // MCH
                if dks
                else 0
            )
            ones_sb = meta_pool.tile([1, P], FP16)
            nc.vector.memset(ones_sb[:], 1.0)
            be_sb = meta_pool.tile([P, 2, W], FP16)
            for s in range(smin, smax):
                for h in range(2):
                    sl = slice(s * MCH, (s + 1) * MCH)
                    pb = psum_pool.tile([P, MCH], F32, name=f"pb{h}_{s}", tag="ps")
                    nc.tensor.matmul(
                        pb[:], ones_sb[:], meta_sb[:, h, sl], start=True, stop=True
                    )
                    nc.vector.tensor_copy(out=be_sb[:, h, sl], in_=pb[:])

            # --- mask strips for k >= KSPLIT on DVE (GpSimd lacks the
            # TensorScalarPtr opcode): mask[p, w] = (b[w] <= t) * (t < e[w]),
            # t = 128k + p.
            dmasks = {}
            for k in dks:
                blo, bhi, hoff = strips[k]
                wlo, whi = blo * P, bhi * P
                wn = whi - wlo
                m2 = m2_pool.tile([P, wn], FP16, name=f"m2_{k}", tag="m2")
                msk = mask_pool.tile([P, wn], FP16, name=f"mask_{k}", tag="mask")
                nc.vector.tensor_scalar(
                    m2[:], be_sb[:, 1, wlo:whi], io_sb[:, k : k + 1], None,
                    mybir.AluOpType.is_gt,
                )
                nc.vector.scalar_tensor_tensor(
                    msk[:], be_sb[:, 0, wlo:whi], io_sb[:, k : k + 1], m2[:],
                    mybir.AluOpType.is_le, mybir.AluOpType.mult,
                )
                dmasks[k] = msk

            # --- block matmuls + ACT evacuation + grouped output DMA.
            og_starts = []
            o0 = 0
            for g in OGROUPS:
                og_starts.append(o0)
                o0 += g
            assert o0 == NBLK

            gi = 0
            os_tile = None
            for i in range(NBLK):
                if i == og_starts[gi]:
                    os_tile = out_pool.tile(
                        [P, OGROUPS[gi], D], FP16, name=f"os{gi}", tag="os"
                    )
                ps = psum_pool.tile([P, D], F32, name=f"ps{i}", tag="ps")
                for k in range(klo[i], khi[i]):
                    blo, bhi, hoff = strips[k]
                    if hoff is not None:
                        lh = m8_sb[:, hoff + (i - blo) * P : hoff + (i - blo + 1) * P]
                    else:
                        lh = dmasks[k][:, (i - blo) * P : (i - blo + 1) * P]
                    cj, cs = k2chunk[k]
                    rh = fhi_tiles[cj][:, cs, :]
                    first = k == klo[i]
                    last = k == khi[i] - 1
                    for n0, nn in ((0, 512), (512, 256)):
                        nc.tensor.matmul(
                            ps[:, n0 : n0 + nn], lh, rh[:, n0 : n0 + nn],
                            start=first, stop=(last and n0 == 512),
                        )
                nc.scalar.mul(
                    out=os_tile[:, i - og_starts[gi], :], in_=ps[:],
                    mul=iv_sb[:, i : i + 1],
                )
                if i == og_starts[gi] + OGROUPS[gi] - 1:
                    eng = nc.scalar if gi % 2 == 0 else nc.sync
                    eng.dma_start(
                        out=out_r[:, og_starts[gi] : i + 1, :], in_=os_tile[:]
                    )
                    gi += 1

    nc.finalize()
    return nc


def _prepare(features, begins, ends):
    feats = np.asarray(features, dtype=np.float32)
    assert feats.shape == (B, T, D), feats.shape
    b = np.clip(np.asarray(begins).astype(np.int64), 0, T - 1)
    e = np.asarray(ends).astype(np.int64)
    # Reference gathers at most MAXWIN tokens starting at b; empty -> count 1.
    e_eff = np.clip(e, b, np.minimum(b + MAXWIN, T))
    counts = np.maximum(e_eff - b, 1).astype(np.float32)
    inv = (1.0 / counts).astype(np.float32)

    bw = b.reshape(B, NBLK, P)
    ew = e_eff.reshape(B, NBLK, P)
    klo_pc = bw.min(-1) // P  # [B, NBLK]
    khi_pc = (np.maximum(ew.max(-1) - 1, bw.min(-1)) // P) + 1
    klo = klo_pc.min(0).astype(int)
    khi = khi_pc.max(0).astype(int)
    khi = np.minimum(np.maximum(khi, klo + 1), NKT)
    klo, khi = list(klo), list(khi)

    strips, hw8 = _strip_layout(klo, khi)

    # shuffle to [P, NKT, D]: partition p holds tokens {p, 128+p, ...}
    hi = np.ascontiguousarray(
        feats.astype(np.float16).reshape(B, NKT, P, D).transpose(0, 2, 1, 3)
    )

    # host fp8 mask head for strips k < KSPLIT (0/1 exact in fp8).
    tok = np.arange(NKT * P).reshape(NKT, P)  # tok[k, p] = 128k + p
    mask8 = np.zeros((B, P, hw8), dtype=FP8NP)
    for k, (blo, bhi, hoff) in strips.items():
        if hoff is None:
            continue
        wlo, whi = blo * P, bhi * P
        t_col = tok[k][:, None]  # [P, 1]
        m = (b[:, wlo:whi][:, None, :] <= t_col) & (
            t_col < e_eff[:, wlo:whi][:, None, :]
        )  # [B, P, wn]
        mask8[:, :, hoff : hoff + (whi - wlo)] = m.astype(FP8NP)

    iota = (
        np.arange(NKT)[None, :] * P + np.arange(P)[:, None] - 2048
    ).astype(np.float32)
    in_maps = []
    for c in range(B):
        metac = np.ascontiguousarray(
            (np.stack([b[c], e_eff[c]]) - 2048).astype(np.float16).reshape(1, 2, W)
        )
        ioiv = np.zeros((P, 64), np.float32)
        ioiv[:, 0:NKT] = iota
        ioiv[:, NKT : NKT + NBLK] = inv[c].reshape(NBLK, P).T
        in_maps.append(
            {"fhi": hi[c], "mask8h": mask8[c], "meta": metac, "ioiv": ioiv}
        )
    return klo, khi, in_maps


def run(features, begins, ends, trace=False):
    """Build + run on 8 NeuronCores; returns (output, BassKernelResults)."""
    klo, khi, in_maps = _prepare(features, begins, ends)
    nc = _build_program(klo, khi)
    res = run_bass_kernel_spmd(nc, in_maps, list(range(B)), trace=trace)
    # out is [P, NBLK, D] fp16 with window w = i*128 + p at [p, i, :]
    out = np.stack(
        [
            np.ascontiguousarray(
                res.results[c]["out"].astype(np.float32).transpose(1, 0, 2)
            ).reshape(W, D)
            for c in range(B)
        ],
        axis=0,
    )
    return out, res


def kernel(features, begins, ends):
    out, _ = run(features, begins, ends, trace=False)
    return out
